# revision 5
# baseline (speedup 1.0000x reference)
"""Trainium2 Bass kernel for nn_AttentionE.

Computes, per sample i:
    s_i   = sum(d_i)                       # d: (N, 6)
    z_ic  = W * s_i * e_ic + b_c           # e: (N, 5), W scalar, b: (5,)
    a_ic  = exp(tanh(z_ic))
    out_ic = e_ic * a_ic / sum_c(a_ic)     # (eps=1e-7 in ref; negligible, denom >= 5/e)

Sharding: data-parallel over the sample axis across 8 NeuronCores.
On-chip layout: each SBUF partition holds a contiguous run of M samples
(rows stay interleaved, [p, m, c]), so DRAM<->SBUF DMAs are fully
contiguous per partition.

Engine split per tile:
  DVE    : sum-of-6 reduce, z = (W*s) bcast-mul e, sum-of-5 reduce, reciprocal
  ScalarE: tanh (per-component, folds bias b_c), exp
  GpSimd : w = a*e, out = w * r_bcast
"""

import sys

import numpy as np

_REPO = "/opt/trn_rl_repo"
if _REPO not in sys.path:
    sys.path.insert(0, _REPO)

from contextlib import ExitStack

import concourse.bacc as bacc
import concourse.bass as bass
import concourse.tile as tile
from concourse import mybir

N_CORES = 8
N_FULL = 4194304
P = 128  # SBUF partitions

# Tunables
M = 512  # samples per partition per tile
BUFS = 3

# Engine assignment for the two multiply stages: "vector" or "gpsimd"
W_ENGINE = "gpsimd"
OUT_ENGINE = "gpsimd"

# test.py can flip this to get profile/exec-time back
TRACE = False
LAST = {}


def build_bass(W: float, bvals, S: int, m: int = M, bufs: int = BUFS):
    """Build the single-core SPMD program: d[S,6], e[S,5] -> out[S,5]."""
    assert S % (P * m) == 0, (S, P, m)
    T = S // (P * m)
    f32 = mybir.dt.float32
    mult = mybir.AluOpType.mult
    add = mybir.AluOpType.add
    X = mybir.AxisListType.X
    ACT = mybir.ActivationFunctionType

    nc = bacc.Bacc("TRN2", debug=False, num_devices=N_CORES)

    # Register the bias values as const APs so activation(bias=<float>) works.
    for i, v in enumerate(dict.fromkeys(float(x) for x in bvals)):
        t_c = nc.alloc_sbuf_tensor(f"const-bias-{i}", [P, 1], f32)
        nc.gpsimd.memset(t_c.ap(), v)
        nc.const_aps.aps[(f32, v)] = t_c.ap()
    nc.all_engine_barrier()

    d_ap = nc.dram_tensor("d", [S, 6], f32, kind="ExternalInput").ap()
    e_ap = nc.dram_tensor("e", [S, 5], f32, kind="ExternalInput").ap()
    o_ap = nc.dram_tensor("out", [S, 5], f32, kind="ExternalOutput").ap()

    # [T, P, m*c] views; per partition the data is one contiguous DRAM run.
    d_v = d_ap.rearrange("(t p m) c -> t p (m c)", t=T, p=P, m=m)
    e_v = e_ap.rearrange("(t p m) c -> t p (m c)", t=T, p=P, m=m)
    o_v = o_ap.rearrange("(t p m) c -> t p (m c)", t=T, p=P, m=m)

    w_eng = {"vector": nc.vector, "gpsimd": nc.gpsimd}[W_ENGINE]
    out_eng = {"vector": nc.vector, "gpsimd": nc.gpsimd}[OUT_ENGINE]

    with tile.TileContext(nc) as tc, ExitStack() as ctx:
        dpool = ctx.enter_context(tc.tile_pool(name="dpool", bufs=bufs))
        epool = ctx.enter_context(tc.tile_pool(name="epool", bufs=bufs))
        zpool = ctx.enter_context(tc.tile_pool(name="zpool", bufs=bufs))
        small = ctx.enter_context(tc.tile_pool(name="small", bufs=bufs))

        for t in range(T):
            dt_ = dpool.tile([P, 6 * m], f32)
            nc.sync.dma_start(out=dt_[:], in_=d_v[t])
            et = epool.tile([P, 5 * m], f32)
            nc.sync.dma_start(out=et[:], in_=e_v[t])

            ev = et[:].rearrange("p (m c) -> p m c", c=5)

            # s = sum over the 6 components of d
            s_t = small.tile([P, m], f32, tag="s")
            nc.vector.tensor_reduce(
                out=s_t[:],
                in_=dt_[:].rearrange("p (m c) -> p m c", c=6),
                axis=X,
                op=add,
            )

            # z = s * e   (s broadcast over the 5 components; W folded into tanh scale)
            z = zpool.tile([P, 5 * m], f32)
            zv = z[:].rearrange("p (m c) -> p m c", c=5)
            s_b = s_t[:].unsqueeze(-1).broadcast_to([P, m, 5])
            nc.vector.tensor_tensor(out=zv, in0=s_b, in1=ev, op=mult)

            # t = tanh(W*z + b_c) per component (scale/bias folded into ACT)
            for c in range(5):
                nc.scalar.activation(
                    out=zv[:, :, c],
                    in_=zv[:, :, c],
                    func=ACT.Tanh,
                    bias=float(bvals[c]),
                    scale=float(W),
                )
            # a = exp(t), full tile
            nc.scalar.activation(out=z[:], in_=z[:], func=ACT.Exp)

            # denom = sum over components of a ; r = 1/denom
            dnm = small.tile([P, m], f32, tag="dnm")
            nc.vector.tensor_reduce(out=dnm[:], in_=zv, axis=X, op=add)
            r = small.tile([P, m], f32, tag="r")
            scr = small.tile([P, m], f32, tag="scr")
            nc.vector.reciprocal_approx_accurate(out=r[:], in_=dnm[:], scratch=scr[:])

            # w = a * e (into e's tile), out = w * r_bcast (into z's tile)
            w_eng.tensor_tensor(out=et[:], in0=z[:], in1=et[:], op=mult)
            r_b = r[:].unsqueeze(-1).broadcast_to([P, m, 5])
            out_eng.tensor_tensor(out=zv, in0=ev, in1=r_b, op=mult)

            nc.sync.dma_start(out=o_v[t], in_=z[:])

    # Legalize: split multi-wait instructions (HW allows 1 wait/inst).
    nc.compile()
    return nc


def kernel(d, e, W, b):
    from concourse.bass_utils import run_bass_kernel_spmd

    d = np.ascontiguousarray(d, dtype=np.float32)
    e = np.ascontiguousarray(e, dtype=np.float32)
    n = d.shape[0]
    assert n % N_CORES == 0
    s = n // N_CORES

    nc = build_bass(float(np.asarray(W).reshape(-1)[0]), np.asarray(b).tolist(), s)

    in_maps = [
        {"d": d[i * s : (i + 1) * s], "e": e[i * s : (i + 1) * s]}
        for i in range(N_CORES)
    ]
    res = run_bass_kernel_spmd(nc, in_maps, list(range(N_CORES)), trace=TRACE)
    LAST["results"] = res
    out = np.concatenate([res.results[i]["out"] for i in range(N_CORES)], axis=0)
    return out.astype(np.float32)


# revision 9
# speedup vs baseline: 1.2461x; 1.2461x over previous
"""Trainium2 Bass kernel for nn_AttentionE.

Computes, per sample i:
    s_i   = sum(d_i)                       # d: (N, 6)
    z_ic  = W * s_i * e_ic + b_c           # e: (N, 5), W scalar, b: (5,)
    a_ic  = exp(tanh(z_ic))
    out_ic = e_ic * a_ic / sum_c(a_ic)     # (eps=1e-7 in ref; negligible, denom >= 5/e)

Sharding: data-parallel over the sample axis across 8 NeuronCores.
On-chip layout: each SBUF partition holds a contiguous run of M samples
(rows stay interleaved, [p, m, c]), so DRAM<->SBUF DMAs are fully
contiguous per partition.

Engine split per tile:
  DVE    : sum-of-6 reduce, z = (W*s) bcast-mul e, sum-of-5 reduce, reciprocal
  ScalarE: tanh (per-component, folds bias b_c), exp
  GpSimd : w = a*e, out = w * r_bcast
"""

import sys

import numpy as np

_REPO = "/opt/trn_rl_repo"
if _REPO not in sys.path:
    sys.path.insert(0, _REPO)

from contextlib import ExitStack

import concourse.bacc as bacc
import concourse.bass as bass
import concourse.tile as tile
from concourse import mybir

N_CORES = 8
N_FULL = 4194304
P = 128  # SBUF partitions

# Tunables
M = 512  # samples per partition per tile
BUFS = 3

# Engine assignment for the two multiply stages: "vector" or "gpsimd"
W_ENGINE = "gpsimd"
OUT_ENGINE = "gpsimd"
# Number of out-stage components (0..5) computed on DVE instead of OUT_ENGINE,
# to balance the DVE and GpSimd pipeline stages.
OUT_SPLIT_K = 0
# Split the d/e input DMAs into two halves so compute starts earlier.
DMA_SPLIT = False
# Pairwise-add reductions instead of tensor_reduce (fewer DVE cycles).
PAIRWISE = False

# test.py can flip this to get profile/exec-time back
TRACE = False
LAST = {}


def build_bass(W: float, bvals, S: int, m: int = M, bufs: int = BUFS):
    """Build the single-core SPMD program: d[S,6], e[S,5] -> out[S,5]."""
    assert S % (P * m) == 0, (S, P, m)
    T = S // (P * m)
    f32 = mybir.dt.float32
    mult = mybir.AluOpType.mult
    add = mybir.AluOpType.add
    X = mybir.AxisListType.X
    ACT = mybir.ActivationFunctionType

    nc = bacc.Bacc("TRN2", debug=False, num_devices=N_CORES)

    # Register the bias values as const APs so activation(bias=<float>) works.
    for i, v in enumerate(dict.fromkeys(float(x) for x in bvals)):
        t_c = nc.alloc_sbuf_tensor(f"const-bias-{i}", [P, 1], f32)
        nc.gpsimd.memset(t_c.ap(), v)
        nc.const_aps.aps[(f32, v)] = t_c.ap()
    nc.all_engine_barrier()

    d_ap = nc.dram_tensor("d", [S, 6], f32, kind="ExternalInput").ap()
    e_ap = nc.dram_tensor("e", [S, 5], f32, kind="ExternalInput").ap()
    o_ap = nc.dram_tensor("out", [S, 5], f32, kind="ExternalOutput").ap()

    # [T, P, m*c] views; per partition the data is one contiguous DRAM run.
    d_v = d_ap.rearrange("(t p m) c -> t p (m c)", t=T, p=P, m=m)
    e_v = e_ap.rearrange("(t p m) c -> t p (m c)", t=T, p=P, m=m)
    o_v = o_ap.rearrange("(t p m) c -> t p (m c)", t=T, p=P, m=m)

    w_eng = {"vector": nc.vector, "gpsimd": nc.gpsimd}[W_ENGINE]
    out_eng = {"vector": nc.vector, "gpsimd": nc.gpsimd}[OUT_ENGINE]

    with tile.TileContext(nc) as tc, ExitStack() as ctx:
        dpool = ctx.enter_context(tc.tile_pool(name="dpool", bufs=bufs))
        epool = ctx.enter_context(tc.tile_pool(name="epool", bufs=bufs))
        zpool = ctx.enter_context(tc.tile_pool(name="zpool", bufs=bufs))
        small = ctx.enter_context(tc.tile_pool(name="small", bufs=bufs))

        for t in range(T):
            dt_ = dpool.tile([P, 6 * m], f32)
            et = epool.tile([P, 5 * m], f32)
            if DMA_SPLIT:
                h = 3 * m  # half of 6m (sample-aligned: 3m = 6*(m/2))
                nc.sync.dma_start(out=dt_[:, :h], in_=d_v[t][:, :h])
                nc.sync.dma_start(out=dt_[:, h:], in_=d_v[t][:, h:])
                he = 5 * (m // 2)
                nc.sync.dma_start(out=et[:, :he], in_=e_v[t][:, :he])
                nc.sync.dma_start(out=et[:, he:], in_=e_v[t][:, he:])
            else:
                nc.sync.dma_start(out=dt_[:], in_=d_v[t])
                nc.sync.dma_start(out=et[:], in_=e_v[t])

            ev = et[:].rearrange("p (m c) -> p m c", c=5)

            # s = sum over the 6 components of d
            s_t = small.tile([P, m], f32, tag="s")
            dv3 = dt_[:].rearrange("p (m c) -> p m c", c=6)
            if PAIRWISE:
                # t3 = d[:, :, 0:3] + d[:, :, 3:6]  (3m elems)
                t3 = small.tile([P, 3 * m], f32, tag="t3")
                t3v = t3[:].rearrange("p (m c) -> p m c", c=3)
                nc.vector.tensor_tensor(
                    out=t3v, in0=dv3[:, :, 0:3], in1=dv3[:, :, 3:6], op=add
                )
                # s = t3[...,0] + t3[...,1] + t3[...,2]
                nc.vector.tensor_tensor(
                    out=s_t[:], in0=t3v[:, :, 0], in1=t3v[:, :, 1], op=add
                )
                nc.vector.tensor_tensor(
                    out=s_t[:], in0=s_t[:], in1=t3v[:, :, 2], op=add
                )
            else:
                nc.vector.tensor_reduce(out=s_t[:], in_=dv3, axis=X, op=add)

            # z = s * e   (s broadcast over the 5 components; W folded into tanh scale)
            z = zpool.tile([P, 5 * m], f32)
            zv = z[:].rearrange("p (m c) -> p m c", c=5)
            s_b = s_t[:].unsqueeze(-1).broadcast_to([P, m, 5])
            nc.vector.tensor_tensor(out=zv, in0=s_b, in1=ev, op=mult)

            # t = tanh(W*z + b_c) per component (scale/bias folded into ACT)
            for c in range(5):
                nc.scalar.activation(
                    out=zv[:, :, c],
                    in_=zv[:, :, c],
                    func=ACT.Tanh,
                    bias=float(bvals[c]),
                    scale=float(W),
                )
            # a = exp(t), full tile
            nc.scalar.activation(out=z[:], in_=z[:], func=ACT.Exp)

            # denom = sum over components of a ; r = 1/denom
            dnm = small.tile([P, m], f32, tag="dnm")
            if PAIRWISE:
                # t2 = a[...,0:2] + a[...,2:4]; dnm = t2[...,0]+t2[...,1]+a[...,4]
                t2 = small.tile([P, 2 * m], f32, tag="t2")
                t2v = t2[:].rearrange("p (m c) -> p m c", c=2)
                nc.vector.tensor_tensor(
                    out=t2v, in0=zv[:, :, 0:2], in1=zv[:, :, 2:4], op=add
                )
                nc.vector.tensor_tensor(
                    out=dnm[:], in0=t2v[:, :, 0], in1=t2v[:, :, 1], op=add
                )
                nc.vector.tensor_tensor(
                    out=dnm[:], in0=dnm[:], in1=zv[:, :, 4], op=add
                )
            else:
                nc.vector.tensor_reduce(out=dnm[:], in_=zv, axis=X, op=add)
            r = small.tile([P, m], f32, tag="r")
            scr = small.tile([P, m], f32, tag="scr")
            nc.vector.reciprocal_approx_accurate(out=r[:], in_=dnm[:], scratch=scr[:])

            # w = a * e (into e's tile), out = w * r_bcast (into z's tile)
            w_eng.tensor_tensor(out=et[:], in0=z[:], in1=et[:], op=mult)
            if OUT_SPLIT_K:
                k = OUT_SPLIT_K
                r_bk = r[:].unsqueeze(-1).broadcast_to([P, m, k])
                nc.vector.tensor_tensor(
                    out=zv[:, :, 0:k], in0=ev[:, :, 0:k], in1=r_bk, op=mult
                )
                r_b = r[:].unsqueeze(-1).broadcast_to([P, m, 5 - k])
                out_eng.tensor_tensor(
                    out=zv[:, :, k:5], in0=ev[:, :, k:5], in1=r_b, op=mult
                )
            else:
                r_b = r[:].unsqueeze(-1).broadcast_to([P, m, 5])
                out_eng.tensor_tensor(out=zv, in0=ev, in1=r_b, op=mult)

            nc.sync.dma_start(out=o_v[t], in_=z[:])

    # Legalize: split multi-wait instructions (HW allows 1 wait/inst).
    nc.compile()
    return nc


def kernel(d, e, W, b):
    from concourse.bass_utils import run_bass_kernel_spmd

    d = np.ascontiguousarray(d, dtype=np.float32)
    e = np.ascontiguousarray(e, dtype=np.float32)
    n = d.shape[0]
    assert n % N_CORES == 0
    s = n // N_CORES

    nc = build_bass(float(np.asarray(W).reshape(-1)[0]), np.asarray(b).tolist(), s)

    in_maps = [
        {"d": d[i * s : (i + 1) * s], "e": e[i * s : (i + 1) * s]}
        for i in range(N_CORES)
    ]
    res = run_bass_kernel_spmd(nc, in_maps, list(range(N_CORES)), trace=TRACE)
    LAST["results"] = res
    out = np.concatenate([res.results[i]["out"] for i in range(N_CORES)], axis=0)
    return out.astype(np.float32)
